# revision 18
# baseline (speedup 1.0000x reference)
"""Trainium2 Bass kernel for nn_AudioDeviceModel (dense_cnn, memory-bound).

The reference model applies a chain of dilated kernel-size-2 convs to a
length-1 sequence with SAME padding.  For dilation d the two taps land at
padded positions 0 and d while the real sample sits at position d//2, so
every conv after the first reduces to its bias; the first conv (dilation 1,
pad_low=0) reduces to tap 0: a dot product of x[b, :] with w1[0, :, 0].
The whole model is therefore

    out[b, j] = (x[b, :] . w1[0, :, 0]) * wd[0, j] + bd_eff[j]
    bd_eff[j] = (b1 + b2 + b3 + b4 + b5) * wd[0, j] + bd[j]

(verified numerically against the jax reference to 1e-7).  This is a pure
memory-bound row-wise dot product over a 512 MiB matrix.

Strategy (v2): data-parallel across 8 NeuronCores (1024 rows each).  The
per-core DMA fabric is 16 engines x ~27 GB/s = ~430 GB/s, so the 64 MiB
x-shard floors at ~156 us; everything else must hide under that stream.
Trace analysis of v1 (201.8 us) showed the DVE at 75% busy (the 16 big
multiply-reduce passes at ~1.07 ns/elem PLUS all PSUM copies + epilogues),
which made it co-critical with DMA and stalled the tail, and 4 MiB of DMA
spent broadcasting v.

v2 changes:
  - DVE runs ONLY the 32 streaming multiply+accumulate passes (~147 us).
  - v broadcast: only the first 2048 columns go over DMA (1 MiB, ready in
    ~3 us so the DVE can start immediately); the remaining 14336 columns
    are replicated on-chip (ones[1,128].T @ v via the idle PE) with the
    PSUM->SBUF copies on the idle Activation engine, not the DVE.
  - epilogue (acc reduce + wd/bd outer product) on gpsimd, not the DVE.
  - nonuniform column chunks [2048, 4096, 5120, 5120]: the small first
    chunk minimizes head latency (DMA v-bcast is small), later chunks are
    large to amortize per-instruction overhead, and per-phase DVE work
    (f * 1.07ns * 8 blocks) stays just under per-phase DMA time.
  - program order interleaves the Activation engine's ring enqueues with
    its copy work so neither ring ever starves.

This container's walrus build only accepts ONE on_wait and ONE on_update
per instruction, while Tile emits multi-wait instructions (kernel-tail
drain, multi-dependency compute ops).  legalize_bir_sync() splits the
extras into standalone EventSemaphore/NoOp instructions on the same engine
(sequencers are in-order, so a wait immediately before an instruction is
equivalent; trailing updates only on non-DMA instructions).
"""

import json

import numpy as np

import concourse.bass as bass
import concourse.mybir as mybir
import concourse.tile as tile
from concourse.bass_utils import run_bass_kernel_spmd

FP32 = mybir.dt.float32

N_CORES = 8
B_FULL = 8192
L = 16384
J = 128
B_CORE = B_FULL // N_CORES  # 1024
P = 128                     # SBUF partitions
N_BB = B_CORE // P          # 8 row-blocks per core

CHUNKS = (2048, 8192, 4096, 2048)   # column phases; sum == L
MM = 512                            # PE broadcast width (one PSUM bank)
VR_PIECE = 4096                     # vrow staging piece (SBUF address space)


def legalize_bir_sync(bir_bytes: bytes) -> bytes:
    """Split >1 on_wait / on_update per instruction for this walrus build."""
    mod = json.loads(bir_bytes)
    for fn in mod["functions"]:
        for bb in fn["blocks"]:
            out = []
            for ins in bb["instructions"]:
                si = ins.get("sync_info")
                waits = (si or {}).get("on_wait") or []
                ups = (si or {}).get("on_update") or []
                if len(waits) > 1:
                    for i, w in enumerate(waits[:-1]):
                        out.append({
                            "debug": ins.get("debug"),
                            "engine": ins["engine"],
                            "ins": [],
                            "outs": [],
                            "name": f"{ins['name']}_lw{i}",
                            "opcode": "EventSemaphore",
                            "sync_info": {"on_update": [], "on_wait": [w]},
                        })
                    si["on_wait"] = [waits[-1]]
                out.append(ins)
                if len(ups) > 1:
                    if ins.get("opcode") == "DMACopy":
                        raise RuntimeError(
                            f"multi-update on DMA {ins['name']} cannot be legalized"
                        )
                    for i, u in enumerate(ups[1:]):
                        out.append({
                            "debug": ins.get("debug"),
                            "engine": ins["engine"],
                            "ins": [],
                            "outs": [],
                            "name": f"{ins['name']}_lu{i}",
                            "opcode": "NoOp",
                            "sync_info": {"on_update": [u], "on_wait": []},
                        })
                    si["on_update"] = [ups[0]]
            bb["instructions"] = out
    return json.dumps(mod).encode()


def install_legalizer(nc):
    orig = nc.to_json_bytes

    def patched():
        return legalize_bir_sync(orig())

    nc.to_json_bytes = patched
    return nc


def build_module() -> bass.Bass:
    n_ch = len(CHUNKS)
    offs = [sum(CHUNKS[:i]) for i in range(n_ch)]
    c0 = CHUNKS[0]
    nc = bass.Bass()
    x_ds = [
        nc.dram_tensor(f"x{bb}", [P, L], FP32, kind="ExternalInput")
        for bb in range(N_BB)
    ]
    v_d = nc.dram_tensor("v", [L], FP32, kind="ExternalInput")
    wd_d = nc.dram_tensor("wdrow", [J], FP32, kind="ExternalInput")
    bd_d = nc.dram_tensor("bdeff", [J], FP32, kind="ExternalInput")
    out_d = nc.dram_tensor("out", [B_CORE, J], FP32, kind="ExternalOutput")

    rings = None  # set below

    with tile.TileContext(nc) as tc:
        with (
            tc.tile_pool(name="consts", bufs=1) as consts,
            tc.tile_pool(name="xp", bufs=3) as xp,
            tc.tile_pool(name="vrp", bufs=1) as vrp,
            tc.tile_pool(name="accp", bufs=2) as accp,
            tc.tile_pool(name="outp", bufs=2) as outp,
            tc.tile_pool(name="psum", bufs=8, space="PSUM") as psum,
        ):
            rings = (nc.sync, nc.scalar)

            # Tiny consts on the gpsimd (SWDGE) ring.
            wd_b = consts.tile([P, J], FP32)
            nc.gpsimd.dma_start(out=wd_b, in_=wd_d[:].unsqueeze(0).partition_broadcast(P))
            bd_b = consts.tile([P, J], FP32)
            nc.gpsimd.dma_start(out=bd_b, in_=bd_d[:].unsqueeze(0).partition_broadcast(P))
            ones = consts.tile([1, P], FP32)
            nc.gpsimd.memset(ones, 1.0)

            # v replicated across partitions, one tile per chunk so each
            # STT depends only on ITS chunk's writers (no false deps).
            v_cs = [
                consts.tile([P, CHUNKS[c]], FP32, name=f"vc{c}", tag=f"vc{c}")
                for c in range(n_ch)
            ]
            # chunk 0 v broadcast AND the first x tile go over SWDGE: the
            # gpsimd queue starts ~6us before the HW rings spool up, so the
            # DVE's first STT can fire at ~10us instead of ~21us.
            nc.gpsimd.dma_start(
                out=v_cs[0],
                in_=v_d[0:c0].unsqueeze(0).partition_broadcast(P),
            )
            x00 = xp.tile([P, c0], FP32, name="x0_0", tag="x")
            nc.gpsimd.dma_start(out=x00, in_=x_ds[0][:, 0:c0])

            def emit_vchunk(c: int):
                # chunk c (c>=1): on-chip replicate.  ones[1,P].T @ v
                # (K=1 so each output is a single product => exact copy);
                # PSUM->SBUF copies on the Activation engine.  vrow is
                # staged through SBUF in <=VR_PIECE sub-pieces.
                f, off = CHUNKS[c], offs[c]
                for sub in range(0, f, VR_PIECE):
                    fs = min(VR_PIECE, f - sub)
                    vr_t = vrp.tile([1, fs], FP32, name=f"vr{c}_{sub}", tag="vr")
                    nc.gpsimd.dma_start(
                        out=vr_t, in_=v_d[off + sub:off + sub + fs].unsqueeze(0)
                    )
                    for k in range(fs // MM):
                        pt = psum.tile([P, MM], FP32, name=f"pt{c}_{sub}_{k}", tag="pt")
                        nc.tensor.matmul(
                            pt, ones, vr_t[:, k * MM:(k + 1) * MM],
                            start=True, stop=True,
                        )
                        nc.scalar.copy(
                            out=v_cs[c][:, sub + k * MM:sub + (k + 1) * MM], in_=pt
                        )

            accs = [
                accp.tile([P, n_ch], FP32, name=f"acc{bb}", tag=f"acc{bb}")
                for bb in range(N_BB)
            ]

            for c in range(n_ch):
                f, off = CHUNKS[c], offs[c]
                # x DMAs for this phase first so both rings stay fed...
                xts = []
                for bb in range(N_BB):
                    if c == 0 and bb == 0:
                        xts.append(x00)  # prefetched over SWDGE above
                        continue
                    x_t = xp.tile([P, f], FP32, name=f"x{c}_{bb}", tag="x")
                    rings[bb % 2].dma_start(out=x_t, in_=x_ds[bb][:, off:off + f])
                    xts.append(x_t)
                # ...then the NEXT phase's v replication (Act engine work
                # lands between this phase's and next phase's enqueues).
                if c + 1 < n_ch:
                    emit_vchunk(c + 1)
                for bb in range(N_BB):
                    # x_t *= v (in place); acc[:, c] = sum over free dim.
                    # The DVE runs ONLY these 32 streaming passes.
                    nc.vector.scalar_tensor_tensor(
                        out=xts[bb],
                        in0=xts[bb],
                        scalar=1.0,
                        in1=v_cs[c],
                        op0=mybir.AluOpType.mult,
                        op1=mybir.AluOpType.mult,
                        accum_out=accs[bb][:, c:c + 1],
                    )
                    if c == n_ch - 1:
                        # epilogue off the DVE: Act reduces acc via
                        # activation's accumulator and forms t*wd via the
                        # per-partition scale operand; Pool adds bd_eff and
                        # writes out.
                        tacc = accp.tile([P, n_ch], FP32, name=f"ta{bb}", tag="ta")
                        t = accp.tile([P, 1], FP32, name=f"t{bb}", tag="t")
                        nc.scalar.activation(
                            out=tacc, in_=accs[bb],
                            func=mybir.ActivationFunctionType.Copy,
                            bias=0.0, scale=1.0, accum_out=t,
                        )
                        o1 = outp.tile([P, J], FP32, name=f"o1_{bb}", tag="o1")
                        nc.scalar.activation(
                            out=o1, in_=wd_b,
                            func=mybir.ActivationFunctionType.Copy,
                            bias=0.0, scale=t,
                        )
                        o_t = outp.tile([P, J], FP32, name=f"o{bb}", tag="o")
                        if bb == N_BB - 1:
                            # last block is the kernel tail: Pool's
                            # tensor_add is ~2.5us, DVE's is ~0.3us, and
                            # the SP ring enqueues faster than SWDGE.
                            nc.vector.tensor_add(out=o_t, in0=o1, in1=bd_b)
                            nc.sync.dma_start(
                                out=out_d[bb * P:(bb + 1) * P, :], in_=o_t
                            )
                        else:
                            nc.gpsimd.tensor_add(out=o_t, in0=o1, in1=bd_b)
                            nc.gpsimd.dma_start(
                                out=out_d[bb * P:(bb + 1) * P, :], in_=o_t
                            )
    install_legalizer(nc)
    return nc


_module_cache: dict = {}


def get_module() -> bass.Bass:
    if "nc" not in _module_cache:
        _module_cache["nc"] = build_module()
    return _module_cache["nc"]


def make_in_maps(inputs: dict) -> list[dict]:
    """Shard the full inputs into one input map per core (pure data parallel
    on the batch dim; tiny weights replicated)."""
    x = np.ascontiguousarray(np.asarray(inputs["x"], dtype=np.float32))
    w1 = np.asarray(inputs["w1"], dtype=np.float32)
    v = np.ascontiguousarray(w1[0, :, 0])
    s0 = float(sum(
        np.asarray(inputs[k], np.float32).reshape(-1)[0]
        for k in ("b1", "b2", "b3", "b4", "b5")
    ))
    wd_row = np.ascontiguousarray(np.asarray(inputs["wd"], np.float32)[0, :])
    bd = np.asarray(inputs["bd"], np.float32).reshape(-1)
    bd_eff = np.ascontiguousarray((s0 * wd_row + bd).astype(np.float32))

    maps = []
    for c in range(N_CORES):
        m = {"v": v, "wdrow": wd_row, "bdeff": bd_eff}
        base = c * B_CORE
        for bb in range(B_CORE // P):
            m[f"x{bb}"] = np.ascontiguousarray(x[base + bb * P:base + (bb + 1) * P])
        maps.append(m)
    return maps


def kernel(**inputs) -> np.ndarray:
    nc = get_module()
    in_maps = make_in_maps(inputs)
    res = run_bass_kernel_spmd(nc, in_maps, core_ids=list(range(N_CORES)))
    return np.concatenate([r["out"] for r in res.results], axis=0)


# revision 21
# speedup vs baseline: 1.1809x; 1.1809x over previous
"""Trainium2 Bass kernel for nn_AudioDeviceModel (dense_cnn, memory-bound).

The reference model applies a chain of dilated kernel-size-2 convs to a
length-1 sequence with SAME padding.  For dilation d the two taps land at
padded positions 0 and d while the real sample sits at position d//2, so
every conv after the first reduces to its bias; the first conv (dilation 1,
pad_low=0) reduces to tap 0: a dot product of x[b, :] with w1[0, :, 0].
The whole model is therefore

    out[b, j] = (x[b, :] . w1[0, :, 0]) * wd[0, j] + bd_eff[j]
    bd_eff[j] = (b1 + b2 + b3 + b4 + b5) * wd[0, j] + bd[j]

(verified numerically against the jax reference to 1e-7).  This is a pure
memory-bound row-wise dot product over a 512 MiB matrix.

Strategy (v2): data-parallel across 8 NeuronCores (1024 rows each).  The
per-core DMA fabric is 16 engines x ~27 GB/s = ~430 GB/s, so the 64 MiB
x-shard floors at ~156 us; everything else must hide under that stream.
Trace analysis of v1 (201.8 us) showed the DVE at 75% busy (the 16 big
multiply-reduce passes at ~1.07 ns/elem PLUS all PSUM copies + epilogues),
which made it co-critical with DMA and stalled the tail, and 4 MiB of DMA
spent broadcasting v.

v2 changes:
  - DVE runs ONLY the 32 streaming multiply+accumulate passes (~147 us).
  - v broadcast: only the first 2048 columns go over DMA (1 MiB, ready in
    ~3 us so the DVE can start immediately); the remaining 14336 columns
    are replicated on-chip (ones[1,128].T @ v via the idle PE) with the
    PSUM->SBUF copies on the idle Activation engine, not the DVE.
  - epilogue (acc reduce + wd/bd outer product) on gpsimd, not the DVE.
  - nonuniform column chunks [2048, 4096, 5120, 5120]: the small first
    chunk minimizes head latency (DMA v-bcast is small), later chunks are
    large to amortize per-instruction overhead, and per-phase DVE work
    (f * 1.07ns * 8 blocks) stays just under per-phase DMA time.
  - program order interleaves the Activation engine's ring enqueues with
    its copy work so neither ring ever starves.

This container's walrus build only accepts ONE on_wait and ONE on_update
per instruction, while Tile emits multi-wait instructions (kernel-tail
drain, multi-dependency compute ops).  legalize_bir_sync() splits the
extras into standalone EventSemaphore/NoOp instructions on the same engine
(sequencers are in-order, so a wait immediately before an instruction is
equivalent; trailing updates only on non-DMA instructions).
"""

import json

import numpy as np

import concourse.bass as bass
import concourse.mybir as mybir
import concourse.tile as tile
from concourse.bass_utils import run_bass_kernel_spmd

FP32 = mybir.dt.float32

N_CORES = 8
B_FULL = 8192
L = 16384
J = 128
B_CORE = B_FULL // N_CORES  # 1024
P = 128                     # SBUF partitions
N_BB = B_CORE // P          # 8 row-blocks per core

CHUNKS = (8192, 8192)               # column phases; sum == L
MM = 512                            # PE broadcast width (one PSUM bank)
VR_PIECE = 4096                     # vrow staging piece (SBUF address space)


def legalize_bir_sync(bir_bytes: bytes) -> bytes:
    """Split >1 on_wait / on_update per instruction for this walrus build."""
    mod = json.loads(bir_bytes)
    for fn in mod["functions"]:
        for bb in fn["blocks"]:
            out = []
            for ins in bb["instructions"]:
                si = ins.get("sync_info")
                waits = (si or {}).get("on_wait") or []
                ups = (si or {}).get("on_update") or []
                if len(waits) > 1:
                    for i, w in enumerate(waits[:-1]):
                        out.append({
                            "debug": ins.get("debug"),
                            "engine": ins["engine"],
                            "ins": [],
                            "outs": [],
                            "name": f"{ins['name']}_lw{i}",
                            "opcode": "EventSemaphore",
                            "sync_info": {"on_update": [], "on_wait": [w]},
                        })
                    si["on_wait"] = [waits[-1]]
                out.append(ins)
                if len(ups) > 1:
                    if ins.get("opcode") == "DMACopy":
                        raise RuntimeError(
                            f"multi-update on DMA {ins['name']} cannot be legalized"
                        )
                    for i, u in enumerate(ups[1:]):
                        out.append({
                            "debug": ins.get("debug"),
                            "engine": ins["engine"],
                            "ins": [],
                            "outs": [],
                            "name": f"{ins['name']}_lu{i}",
                            "opcode": "NoOp",
                            "sync_info": {"on_update": [u], "on_wait": []},
                        })
                    si["on_update"] = [ups[0]]
            bb["instructions"] = out
    return json.dumps(mod).encode()


def install_legalizer(nc):
    orig = nc.to_json_bytes

    def patched():
        return legalize_bir_sync(orig())

    nc.to_json_bytes = patched
    return nc


def build_module() -> bass.Bass:
    n_ch = len(CHUNKS)
    offs = [sum(CHUNKS[:i]) for i in range(n_ch)]
    c0 = CHUNKS[0]
    nc = bass.Bass()
    x_ds = [
        nc.dram_tensor(f"x{bb}", [P, L], FP32, kind="ExternalInput")
        for bb in range(N_BB)
    ]
    v_d = nc.dram_tensor("v", [L], FP32, kind="ExternalInput")
    wd_d = nc.dram_tensor("wdrow", [J], FP32, kind="ExternalInput")
    bd_d = nc.dram_tensor("bdeff", [J], FP32, kind="ExternalInput")
    out_d = nc.dram_tensor("out", [B_CORE, J], FP32, kind="ExternalOutput")

    rings = None  # set below

    with tile.TileContext(nc) as tc:
        with (
            tc.tile_pool(name="consts", bufs=1) as consts,
            tc.tile_pool(name="xp", bufs=3) as xp,
            tc.tile_pool(name="vrp", bufs=1) as vrp,
            tc.tile_pool(name="accp", bufs=2) as accp,
            tc.tile_pool(name="outp", bufs=2) as outp,
            tc.tile_pool(name="psum", bufs=8, space="PSUM") as psum,
        ):
            rings = (nc.sync, nc.scalar)

            # Tiny consts on the gpsimd (SWDGE) ring.
            wd_b = consts.tile([P, J], FP32)
            nc.gpsimd.dma_start(out=wd_b, in_=wd_d[:].unsqueeze(0).partition_broadcast(P))
            bd_b = consts.tile([P, J], FP32)
            nc.gpsimd.dma_start(out=bd_b, in_=bd_d[:].unsqueeze(0).partition_broadcast(P))
            ones = consts.tile([1, P], FP32)
            nc.gpsimd.memset(ones, 1.0)

            # v replicated across partitions, one tile per chunk so each
            # STT depends only on ITS chunk's writers (no false deps).
            v_cs = [
                consts.tile([P, CHUNKS[c]], FP32, name=f"vc{c}", tag=f"vc{c}")
                for c in range(n_ch)
            ]
            # chunk 0: stride-0 DMA broadcast (4 MiB of bus traffic) split
            # across both rings ahead of their phase-0 x tiles.  This costs
            # ~9us of bus time but keeps phase 0's DVE demand well under
            # its DMA time, which is what keeps the pipeline out of the
            # slot-gated regime (measured: SWDGE starts LATER than the HW
            # rings, so prefetching via gpsimd does not work).
            h = c0 // 2
            for r in range(2):
                rings[r].dma_start(
                    out=v_cs[0][:, r * h:(r + 1) * h],
                    in_=v_d[r * h:(r + 1) * h].unsqueeze(0).partition_broadcast(P),
                )

            def emit_vchunk(c: int):
                # chunk c (c>=1): on-chip replicate.  ones[1,P].T @ v
                # (K=1 so each output is a single product => exact copy);
                # PSUM->SBUF copies on the Activation engine.  vrow is
                # staged through SBUF in <=VR_PIECE sub-pieces.
                f, off = CHUNKS[c], offs[c]
                for sub in range(0, f, VR_PIECE):
                    fs = min(VR_PIECE, f - sub)
                    vr_t = vrp.tile([1, fs], FP32, name=f"vr{c}_{sub}", tag="vr")
                    nc.gpsimd.dma_start(
                        out=vr_t, in_=v_d[off + sub:off + sub + fs].unsqueeze(0)
                    )
                    for k in range(fs // MM):
                        pt = psum.tile([P, MM], FP32, name=f"pt{c}_{sub}_{k}", tag="pt")
                        nc.tensor.matmul(
                            pt, ones, vr_t[:, k * MM:(k + 1) * MM],
                            start=True, stop=True,
                        )
                        nc.scalar.copy(
                            out=v_cs[c][:, sub + k * MM:sub + (k + 1) * MM], in_=pt
                        )

            accs = [
                accp.tile([P, n_ch], FP32, name=f"acc{bb}", tag=f"acc{bb}")
                for bb in range(N_BB)
            ]

            for c in range(n_ch):
                f, off = CHUNKS[c], offs[c]
                # x DMAs for this phase first so both rings stay fed...
                xts = []
                for bb in range(N_BB):
                    x_t = xp.tile([P, f], FP32, name=f"x{c}_{bb}", tag="x")
                    rings[bb % 2].dma_start(out=x_t, in_=x_ds[bb][:, off:off + f])
                    xts.append(x_t)
                # ...then the NEXT phase's v replication (Act engine work
                # lands between this phase's and next phase's enqueues).
                if c + 1 < n_ch:
                    emit_vchunk(c + 1)
                for bb in range(N_BB):
                    # x_t *= v (in place); acc[:, c] = sum over free dim.
                    # The DVE runs ONLY these 32 streaming passes.
                    nc.vector.scalar_tensor_tensor(
                        out=xts[bb],
                        in0=xts[bb],
                        scalar=1.0,
                        in1=v_cs[c],
                        op0=mybir.AluOpType.mult,
                        op1=mybir.AluOpType.mult,
                        accum_out=accs[bb][:, c:c + 1],
                    )
                    if c == n_ch - 1:
                        # epilogue off the DVE: Act reduces acc via
                        # activation's accumulator and forms t*wd via the
                        # per-partition scale operand; Pool adds bd_eff and
                        # writes out.
                        tacc = accp.tile([P, n_ch], FP32, name=f"ta{bb}", tag="ta")
                        t = accp.tile([P, 1], FP32, name=f"t{bb}", tag="t")
                        nc.scalar.activation(
                            out=tacc, in_=accs[bb],
                            func=mybir.ActivationFunctionType.Copy,
                            bias=0.0, scale=1.0, accum_out=t,
                        )
                        o1 = outp.tile([P, J], FP32, name=f"o1_{bb}", tag="o1")
                        nc.scalar.activation(
                            out=o1, in_=wd_b,
                            func=mybir.ActivationFunctionType.Copy,
                            bias=0.0, scale=t,
                        )
                        o_t = outp.tile([P, J], FP32, name=f"o{bb}", tag="o")
                        if bb == N_BB - 1:
                            # last block is the kernel tail: Pool's
                            # tensor_add is ~2.5us, DVE's is ~0.3us, and
                            # the SP ring enqueues faster than SWDGE.
                            nc.vector.tensor_add(out=o_t, in0=o1, in1=bd_b)
                            nc.sync.dma_start(
                                out=out_d[bb * P:(bb + 1) * P, :], in_=o_t
                            )
                        else:
                            nc.gpsimd.tensor_add(out=o_t, in0=o1, in1=bd_b)
                            nc.gpsimd.dma_start(
                                out=out_d[bb * P:(bb + 1) * P, :], in_=o_t
                            )
    install_legalizer(nc)
    return nc


_module_cache: dict = {}


def get_module() -> bass.Bass:
    if "nc" not in _module_cache:
        _module_cache["nc"] = build_module()
    return _module_cache["nc"]


def make_in_maps(inputs: dict) -> list[dict]:
    """Shard the full inputs into one input map per core (pure data parallel
    on the batch dim; tiny weights replicated)."""
    x = np.ascontiguousarray(np.asarray(inputs["x"], dtype=np.float32))
    w1 = np.asarray(inputs["w1"], dtype=np.float32)
    v = np.ascontiguousarray(w1[0, :, 0])
    s0 = float(sum(
        np.asarray(inputs[k], np.float32).reshape(-1)[0]
        for k in ("b1", "b2", "b3", "b4", "b5")
    ))
    wd_row = np.ascontiguousarray(np.asarray(inputs["wd"], np.float32)[0, :])
    bd = np.asarray(inputs["bd"], np.float32).reshape(-1)
    bd_eff = np.ascontiguousarray((s0 * wd_row + bd).astype(np.float32))

    maps = []
    for c in range(N_CORES):
        m = {"v": v, "wdrow": wd_row, "bdeff": bd_eff}
        base = c * B_CORE
        for bb in range(B_CORE // P):
            m[f"x{bb}"] = np.ascontiguousarray(x[base + bb * P:base + (bb + 1) * P])
        maps.append(m)
    return maps


def kernel(**inputs) -> np.ndarray:
    nc = get_module()
    in_maps = make_in_maps(inputs)
    res = run_bass_kernel_spmd(nc, in_maps, core_ids=list(range(N_CORES)))
    return np.concatenate([r["out"] for r in res.results], axis=0)
